# revision 20
# baseline (speedup 1.0000x reference)
"""Trainium2 Bass kernel for nn_CausalAttention (gated-resnet q/k/v projections
+ causal attention). Data-parallel over batch: 8 batches -> 8 NeuronCores.

Per-core computation (batch b), all fp32 storage:
  x_q = query[b] (C=256, S=1024)   x_k = key[b] (256, 1024)
  branch(p, x): e  = elu(x)
                h1 = W1 @ e + b1 ; e1 = elu(h1)
                h2 = W2 @ e1 + b2 ; a, g = split(h2)
                gr = x + a * sigmoid(g)
                o  = Wn @ gr + bn          (512, 1024) channel-major
  q = branch(q, x_q); k = branch(k, x_k); v = branch(v, x_k)
  att view: X_att[s, d] = X_cm[s//2, (s%2)*512 + d]  (flat reinterpretation)
  per head n (d = 64n..64n+63):
    scoresT[s2, s1] = sum_d K_att[s2,d] Q_att[s1,d]   (s2 causal blocks)
    eT = exp(scoresT/sqrt(512)) with strict-lower mask (s2 < s1)
    outT[vs, s1] = sum_s2 V_att[s2, 64n+vs] * eT[s2, s1] ; l[s1] = sum_s2 eT
    final[64n+vs, s1] = outT[vs, s1] / l[s1]   (row 0 of l patched to 1)

v2: engine-rebalanced + software-pipelined:
  - branches issued stage-interleaved (h1 v,k,q; e1 v,k,q; ...) so PE
    matmuls of one branch overlap DVE/ACT work of another
  - elu combine / glu mult / gr add / masks moved to gpsimd (was idle)
  - v_aug built by direct SBUF->SBUF DMA (no DRAM roundtrip)
  - softmax normalize: reciprocal (DVE) -> gpsimd partition_broadcast ->
    DVE multiply (no DRAM roundtrips, no big psum->sbuf copy)
"""

import os
import sys
import numpy as np

sys.path.insert(0, "/opt/trn_rl_repo")

C = 256
S = 1024
D = 512
NH = 8
KS = 64
VS = 64
SCALE = 1.0 / float(np.sqrt(512.0))
N_CORES = 8

CFG = {
    "mm_dtype": "bfloat16",  # "float32" | "bfloat16"
    # gpsimd only supports plain tensor_tensor (no scalar-imm ops)
    "elu_combine_engine": "gpsimd",  # dst = me' + r   (me' = min(e,1)-1)
    "glu_mult_engine": "vector",     # u = ha*(1+tg)   (stt needs V)
    "gr_add_engine": "gpsimd",       # gr = u + x
    "mask_engine": "gpsimd",         # eT diag *= mask01
    "fin_engine": "vector",          # fin = pv * rb
    "bcast": "gpsimd",               # rb broadcast: "gpsimd" | "dma"
}


def _register_custom_dve_ops():
    """Register fused DVE ops (runtime extension of dve_ops.OPS):
      ELU_FUSED_ANT: out = select(in1+s0 > 0, in1+s0, min(in0,1)-1)
                     (in0 = exp(in1+s0) from ScalarE; elu in one DVE pass)
      GLU_FUSED_ANT: out = (in1+s0) * (in0+1) * 0.5
                     (in0 = tanh(0.5 g + 0.5 b2g); gated half-sum in one pass)
    """
    from concourse import dve_ops as DO
    from concourse.dve_spec import (
        Spec, Src0, Src1, C0, C1, Zero, One, minn, select, lower,
        _has_src1 as has_src1,
    )
    from concourse.dve_uop import DveOpSpec
    import numpy as np

    if any(op.name == "ELU_FUSED_ANT" for op in DO.OPS):
        return {op.name: op for op in DO.OPS}

    def mk(name, spec):
        opcode = DO._CUSTOM_DVE_ROW_BASE + len(DO.OPS)
        shas = {}
        for ver in ("v3", "v4"):
            s = DveOpSpec(name=name, opcode=opcode,
                          uops=lower(spec, ver=ver), rd1_en=has_src1(spec))
            shas[ver] = s.sha(ver)
        op = DO.DveOp(name, spec, subdim=False, uops_sha=shas)
        DO.OPS.append(op)
        DO.CUSTOM_DVE_SPECS[name] = spec
        DO._SUB_OPCODE_FOR_NAME[name] = opcode
        return op

    t = Src1 + C0
    elu = mk("ELU_FUSED_ANT", Spec(
        body=select(t > Zero, t, minn(Src0, One) - One),
        reference=lambda in0, in1, s0, s1, imm2: np.where(
            in1 + s0 > 0, in1 + s0, np.minimum(in0, 1.0) - 1.0
        ).astype(np.float32),
    ))
    glu = mk("GLU_FUSED_ANT", Spec(
        body=(Src1 + C0) * ((Src0 + One) * C1),
        reference=lambda in0, in1, s0, s1, imm2: (
            (in1 + s0) * (in0 + 1.0) * s1
        ).astype(np.float32),
    ))
    return {"ELU_FUSED_ANT": elu, "GLU_FUSED_ANT": glu}


def _split_psum_ranges(a, b, max_n=512):
    """Split [a, b) psum column range into chunks that don't cross 512-col
    bank boundaries and are <= max_n wide."""
    out = []
    while a < b:
        nxt = min(b, ((a // 512) + 1) * 512, a + max_n)
        out.append((a, nxt))
        a = nxt
    return out


def build_program(cfg=CFG):
    from contextlib import ExitStack

    import concourse.bacc as bacc
    import concourse.bass as bass
    import concourse.tile as tile
    from concourse import mybir
    from concourse.alu_op_type import AluOpType as Op

    f32 = mybir.dt.float32
    mmdt = getattr(mybir.dt, cfg["mm_dtype"])
    mdt = mmdt
    AF = mybir.ActivationFunctionType

    fused = _register_custom_dve_ops()
    ELU_OP = fused["ELU_FUSED_ANT"]
    GLU_OP = fused["GLU_FUSED_ANT"]

    nc = bacc.Bacc("TRN2", target_bir_lowering=False, debug=False,
                   num_devices=N_CORES)

    # ---------------- DRAM parameters ----------------
    idt = mybir.dt.bfloat16 if cfg["mm_dtype"] == "bfloat16" else f32
    query = nc.dram_tensor("query", [C, S], idt, kind="ExternalInput").ap()
    key = nc.dram_tensor("key", [C, S], idt, kind="ExternalInput").ap()
    wT = {}
    bias = {}
    wdt = mdt if mdt == mybir.dt.bfloat16 else f32
    for p in ("q", "k", "v"):
        wT[p, 1] = nc.dram_tensor(f"{p}_w1T", [C, C], wdt, kind="ExternalInput").ap()
        wT[p, 2] = nc.dram_tensor(f"{p}_w2T", [C, 2 * C], wdt, kind="ExternalInput").ap()
        wT[p, "n"] = nc.dram_tensor(f"{p}_wnT", [C, D], wdt, kind="ExternalInput").ap()
        bias[p, 1] = nc.dram_tensor(f"{p}_b1", [C], f32, kind="ExternalInput").ap()
        bias[p, 2] = nc.dram_tensor(f"{p}_b2", [2 * C], f32, kind="ExternalInput").ap()
        bias[p, "n"] = nc.dram_tensor(f"{p}_bn", [D], f32, kind="ExternalInput").ap()
    out_d = nc.dram_tensor("out", [D, S], f32, kind="ExternalOutput").ap()

    def eng(name):
        return getattr(nc, name)

    BR = ("v", "k", "q")  # issue order within stages

    with tile.TileContext(nc) as tc, ExitStack() as ctx:
        # ------------- pools -------------
        persist = ctx.enter_context(tc.tile_pool(name="persist", bufs=1))
        psum_main = ctx.enter_context(tc.tile_pool(name="psum_main", bufs=2, space="PSUM"))
        psum_pv = ctx.enter_context(tc.tile_pool(name="psum_pv", bufs=2, space="PSUM"))
        wk = ctx.enter_context(tc.tile_pool(name="wk", bufs=5))
        big = ctx.enter_context(tc.tile_pool(name="big", bufs=1))
        eT_pool = ctx.enter_context(tc.tile_pool(name="eT", bufs=3))
        att_small = ctx.enter_context(tc.tile_pool(name="att_small", bufs=2))

        # persistent tiles
        xq = persist.tile([128, 2, S], idt)
        xk = persist.tile([128, 2, S], idt)
        eluq = persist.tile([128, 2, S], mdt)
        eluk = persist.tile([128, 2, S], mdt)
        qT_m = persist.tile([128, 4, S], mdt)   # Q^T_att: [dd%128, dd//128, s]
        kT_m = persist.tile([128, 4, S], mdt)
        v_aug = persist.tile([128, 8, NH, VS + 1], mdt)  # [s%128, s//128, n, vs|1]

        # PE warm-up: back-to-back matmuls on scratch data during DMA phase
        warm = persist.tile([128, 512], mdt, name="warm")
        nc.vector.memset(warm, 0.5)
        wps = psum_main.tile([128, 1024], f32, tag="pm", name="wps")
        for _ in range(18):
            nc.tensor.matmul(wps[:, 0:512], lhsT=warm[:, 0:128],
                             rhs=warm, start=True, stop=True)

        # inputs
        for cc in range(2):
            nc.sync.dma_start(out=xk[:, cc, :], in_=key[cc * 128:(cc + 1) * 128, :])
            nc.sync.dma_start(out=xq[:, cc, :], in_=query[cc * 128:(cc + 1) * 128, :])
        bnb = {"q": persist.tile([128, D], f32, name="bnb_q"),
               "k": persist.tile([128, D], f32, name="bnb_k")}
        for p in ("q", "k"):
            bn_ap = bias[p, "n"]
            bn_bcast = bass.AP(tensor=bn_ap.tensor, offset=bn_ap.offset,
                               ap=[[0, 128]] + list(bn_ap.ap))
            nc.sync.dma_start(out=bnb[p], in_=bn_bcast)
        # v_aug ones column (independent of everything; do during DMA phase)
        for j in range(8):
            nc.gpsimd.memset(v_aug[:, j, :, VS:VS + 1], 1.0)

        # ------- weights for all branches upfront (persistent) -------
        w1 = {}
        w2 = {}
        wn = {}
        b1 = {}
        b2 = {}
        b2h = {}
        for p in BR:
            w1[p] = persist.tile([128, 2, C], mdt, name=f"w1_{p}")
            w2[p] = persist.tile([128, 2, 2 * C], mdt, name=f"w2_{p}")
            wn[p] = persist.tile([128, 2, D], mdt, name=f"wn_{p}")

            def wcast(ap):
                return ap if ap.dtype == mdt else ap.bitcast(mdt)
            for kc in range(2):
                nc.sync.dma_start(out=w1[p][:, kc, :], in_=wcast(wT[p, 1][kc * 128:(kc + 1) * 128, :]))
                nc.sync.dma_start(out=w2[p][:, kc, :], in_=wcast(wT[p, 2][kc * 128:(kc + 1) * 128, :]))
                nc.sync.dma_start(out=wn[p][:, kc, :], in_=wcast(wT[p, "n"][kc * 128:(kc + 1) * 128, :]))
            b1[p] = persist.tile([128, 2], f32, name=f"b1_{p}")
            b2[p] = persist.tile([128, 4], f32, name=f"b2_{p}")
            b2h[p] = persist.tile([128, 4], f32, name=f"b2h_{p}")
            nc.sync.dma_start(out=b1[p], in_=bias[p, 1].rearrange("(kc p) -> p kc", p=128))
            nc.sync.dma_start(out=b2[p], in_=bias[p, 2].rearrange("(kc p) -> p kc", p=128))
            nc.vector.tensor_scalar_mul(b2h[p], b2[p], 0.5)
        bnv = persist.tile([128, 4], f32, name="bnv")
        nc.sync.dma_start(out=bnv, in_=bias["v", "n"].rearrange("(kc p) -> p kc", p=128))

        # ---------------- branch compute, stage-interleaved ----------------
        # elu(x) for inputs: xk first (feeds v and k), then xq
        def elu_chunk(dst, src_ap, bias_col=None):
            """dst = elu(src [+ bias]) for a (128, S) chunk. src may be psum.
            scalar: e = exp(src + b); vector (fused): dst =
            select(src+b > 0, src+b, min(e,1)-1)."""
            e = wk.tile([128, S], mdt, tag="wke")
            if bias_col is None:
                nc.scalar.activation(e, src_ap, AF.Exp)
                nc.vector._custom_dve(ELU_OP, out=dst, in0=e, in1=src_ap,
                                      s0=0.0)
            else:
                nc.scalar.activation(e, src_ap, AF.Exp, bias=bias_col)
                nc.vector._custom_dve(ELU_OP, out=dst, in0=e, in1=src_ap,
                                      s0=bias_col)

        for cc in range(2):
            elu_chunk(eluk[:, cc, :], xk[:, cc, :])
        for cc in range(2):
            elu_chunk(eluq[:, cc, :], xq[:, cc, :])

        elu3 = {"v": eluk, "k": eluk, "q": eluq}
        x3 = {"v": xk, "k": xk, "q": xq}

        # --- stage 1: h1 = W1 @ elu(x) + b1 ; e1 = elu(h1) ---
        e1 = {p: big.tile([128, 2, S], mdt, name=f"e1_{p}") for p in BR}
        for p in BR:
            for mc in range(2):
                ps = psum_main.tile([128, 1024], f32, tag="pm")
                h1 = ps[:, 0:S]
                for nk in range(2):
                    for kc in range(2):
                        nc.tensor.matmul(
                            h1[:, nk * 512:(nk + 1) * 512],
                            lhsT=w1[p][:, kc, mc * 128:(mc + 1) * 128],
                            rhs=elu3[p][:, kc, nk * 512:(nk + 1) * 512],
                            start=(kc == 0), stop=(kc == 1))
                elu_chunk(e1[p][:, mc, :], h1, bias_col=b1[p][:, mc:mc + 1])

        # --- stage 2: h2 = W2 @ e1 + b2 ; gr = x + 0.5(a+b2a)(1+tanh(0.5(g+b2g))) ---
        gr = {p: big.tile([128, 2, S], mdt, name=f"gr_{p}") for p in BR}
        for p in BR:
            for cc in range(2):
                ps_a = psum_main.tile([128, 1024], f32, tag="pm")
                a_raw = ps_a[:, 0:S]
                for nk in range(2):
                    for kc in range(2):
                        nc.tensor.matmul(
                            a_raw[:, nk * 512:(nk + 1) * 512],
                            lhsT=w2[p][:, kc, cc * 128:(cc + 1) * 128],
                            rhs=e1[p][:, kc, nk * 512:(nk + 1) * 512],
                            start=(kc == 0), stop=(kc == 1))
                ps_g = psum_main.tile([128, 1024], f32, tag="pm")
                g_raw = ps_g[:, 0:S]
                for nk in range(2):
                    for kc in range(2):
                        nc.tensor.matmul(
                            g_raw[:, nk * 512:(nk + 1) * 512],
                            lhsT=w2[p][:, kc, (2 + cc) * 128:(3 + cc) * 128],
                            rhs=e1[p][:, kc, nk * 512:(nk + 1) * 512],
                            start=(kc == 0), stop=(kc == 1))
                tg = wk.tile([128, S], mdt, tag="wke")
                u = wk.tile([128, S], mdt, tag="wku")
                nc.scalar.activation(tg, g_raw, AF.Tanh,
                                     bias=b2h[p][:, 2 + cc:3 + cc], scale=0.5)
                # u = (a + b2a) * 0.5 * (1 + tg)   (fused DVE)
                nc.vector._custom_dve(GLU_OP, out=u, in0=tg, in1=a_raw,
                                      s0=b2[p][:, cc:cc + 1], s1=0.5)
                eng(cfg["gr_add_engine"]).tensor_tensor(
                    gr[p][:, cc, :], u, x3[p][:, cc, :], Op.add)

        # --- stage 3: nin ---
        # v first (feeds v_aug via sbuf->sbuf DMA), then k/q interleaved by
        # hw-block pairs so tp chunks complete in order 0,1,2,3.
        v_sb = big.tile([128, 4, S], mdt, name="v_sb")
        for mc in range(4):
            ps = psum_main.tile([128, 1024], f32, tag="pm")
            vo = ps[:, 0:S]
            for nk in range(2):
                for kc in range(2):
                    nc.tensor.matmul(
                        vo[:, nk * 512:(nk + 1) * 512],
                        lhsT=wn["v"][:, kc, mc * 128:(mc + 1) * 128],
                        rhs=gr["v"][:, kc, nk * 512:(nk + 1) * 512],
                        start=(kc == 0), stop=(kc == 1))
            nc.vector.tensor_scalar(v_sb[:, mc, :], vo, bnv[:, mc:mc + 1], 0.0,
                                    Op.add, Op.add)
            # v_aug[j][p2, n, u] = V_att[128j+p2, 64n+u]; V_att[s, d] =
            # v_cm[s//2, (s%2)*512 + d].  j blocks 2*mc, 2*mc+1 live in chunk mc.
            for j in (2 * mc, 2 * mc + 1):
                src = v_sb[64 * (j % 2):64 * (j % 2) + 64, mc, :]
                src = src.rearrange("c (h n u) -> c h n u", h=2, n=NH)
                nc.sync.dma_start(out=v_aug[:, j, :, 0:VS], in_=src)

        def nin_T(p, hw_p):
            tgt = qT_m if p == "q" else kT_m
            ps = psum_main.tile([128, 1024], f32, tag="pm")
            oT = ps[:, 0:D]
            for kc in range(2):
                nc.tensor.matmul(
                    oT,
                    lhsT=gr[p][:, kc, hw_p * 128:(hw_p + 1) * 128],
                    rhs=wn[p][:, kc, :],
                    start=(kc == 0), stop=(kc == 1))
            tp, jj = hw_p % 4, hw_p // 4
            nc.vector.tensor_tensor(tgt[:, tp, jj::2], oT, bnb[p], Op.add)

        for hw_p in (0, 4, 1, 5, 2, 6, 3, 7):
            nin_T("k", hw_p)
            nin_T("q", hw_p)

        # ---------------- attention ----------------
        # scores psum groups (each <= 1024 cols = 2 banks)
        GROUPS = [(0,), (1, 7), (2, 6), (3, 5), (4,)]
        G = {}
        off = 0
        for grp in GROUPS:
            for j in grp:
                G[j] = off
                off += S - 128 * j

        for n in range(NH):
            tp, po = n // 2, 64 * (n % 2)
            eT = eT_pool.tile([128, 4608], mdt, tag="eT")
            for grp in GROUPS:
                glen = sum(S - 128 * j for j in grp)
                gbase = G[grp[0]]
                ps = psum_main.tile([128, 1024], f32, tag="pm")
                for j in grp:
                    off = G[j] - gbase
                    lhsT = kT_m[po:po + 64, tp, 128 * j:128 * (j + 1)]
                    for s1a, s1b in _split_psum_ranges(off, off + (S - 128 * j)):
                        nc.tensor.matmul(
                            ps[:, s1a:s1b],
                            lhsT=lhsT,
                            rhs=qT_m[po:po + 64, tp,
                                     128 * j + (s1a - off):128 * j + (s1b - off)],
                            start=True, stop=True)
                nc.scalar.activation(eT[:, gbase:gbase + glen],
                                     ps[:, 0:glen], AF.Exp, scale=SCALE)
                for j in grp:
                    # strict-lower mask on the diagonal block, in place:
                    # keep where t1 - t2 - 1 >= 0 else 0
                    nc.gpsimd.affine_select(
                        out=eT[:, G[j]:G[j] + 128], in_=eT[:, G[j]:G[j] + 128],
                        compare_op=Op.is_ge, fill=0.0,
                        base=-1, pattern=[[1, 128]], channel_multiplier=-1)

            pv = psum_pv.tile([VS + 1, 1024], f32, tag="pv")
            for c in range(2):
                jmax = 3 if c == 0 else 7
                for j in range(jmax + 1):
                    s1a = max(512 * c, 128 * j)
                    s1b = 512 * (c + 1)
                    nc.tensor.matmul(
                        pv[:, s1a:s1b],
                        lhsT=v_aug[:, j, n, :],
                        rhs=eT[:, G[j] + (s1a - 128 * j):G[j] + (s1b - 128 * j)],
                        start=(j == 0), stop=(j == jmax))
            nc.vector.memset(pv[VS:VS + 1, 0:1], 1.0)
            # normalize: 1/l -> broadcast to 64 partitions -> multiply
            lrow = att_small.tile([1, 1024], f32, tag="lrow")
            nc.vector.tensor_copy(lrow, pv[VS:VS + 1, :])
            rrow = att_small.tile([1, 1024], f32, tag="rrow")
            nc.vector.reciprocal_approx_fast(rrow, lrow)
            rb = att_small.tile([VS, 1024], f32, tag="rb")
            if cfg["bcast"] == "gpsimd":
                nc.gpsimd.partition_broadcast(rb, rrow, channels=VS)
            else:
                rsrc = bass.AP(tensor=rrow.tensor, offset=rrow.offset,
                               ap=[[0, VS]] + list(rrow.ap)[1:])
                nc.sync.dma_start(out=rb, in_=rsrc)
            fin = att_small.tile([VS, 1024], f32, tag="fin")
            eng(cfg["fin_engine"]).tensor_tensor(
                fin, pv[0:VS, :], rb, Op.mult)
            nc.sync.dma_start(out=out_d[VS * n:VS * (n + 1), :], in_=fin)

    nc.compile()
    return nc


_CACHE = {}


def _get_program(cfg_key=None):
    key = cfg_key or "default"
    if key not in _CACHE:
        _CACHE[key] = build_program(CFG)
    return _CACHE[key]


def make_in_map(inp, b):
    """Per-core input dict for batch b (weights host-transposed/cast)."""
    if CFG["mm_dtype"] == "bfloat16":
        import ml_dtypes
        wt = np.dtype(ml_dtypes.bfloat16)
    else:
        wt = np.float32
    m = {
        "query": np.ascontiguousarray(inp["query"][b].reshape(C, S)).astype(wt),
        "key": np.ascontiguousarray(inp["key"][b].reshape(C, S)).astype(wt),
    }
    for p in ("q", "k", "v"):
        m[f"{p}_w1T"] = np.ascontiguousarray(inp[f"{p}_gr_w1"].T).astype(wt)
        m[f"{p}_w2T"] = np.ascontiguousarray(inp[f"{p}_gr_w2"].T).astype(wt)
        m[f"{p}_wnT"] = np.ascontiguousarray(inp[f"{p}_nin_w"].T).astype(wt)
        m[f"{p}_b1"] = inp[f"{p}_gr_b1"]
        m[f"{p}_b2"] = inp[f"{p}_gr_b2"]
        m[f"{p}_bn"] = inp[f"{p}_nin_b"]
    return m


def kernel(**inputs):
    from concourse.bass_utils import run_bass_kernel_spmd

    nc = _get_program()
    inp = {k: np.asarray(v, dtype=np.float32) for k, v in inputs.items()}

    in_maps = [make_in_map(inp, b) for b in range(N_CORES)]

    trace = bool(int(os.environ.get("BASS_KERNEL_TRACE", "0")))
    res = run_bass_kernel_spmd(nc, in_maps, core_ids=list(range(N_CORES)),
                               trace=trace)
    LAST_RUN["exec_time_ns"] = getattr(res, "exec_time_ns", None)
    LAST_RUN["results"] = res
    out = np.stack([res.results[i]["out"].reshape(D, 32, 32)
                    for i in range(N_CORES)])
    return out.astype(np.float32)


LAST_RUN = {}


if __name__ == "__main__":
    nc = build_program()
    print("compiled OK")


# revision 27
# speedup vs baseline: 1.1597x; 1.1597x over previous
"""Trainium2 Bass kernel for nn_CausalAttention (gated-resnet q/k/v projections
+ causal attention). Data-parallel over batch: 8 batches -> 8 NeuronCores.

Per-core computation (batch b), all fp32 storage:
  x_q = query[b] (C=256, S=1024)   x_k = key[b] (256, 1024)
  branch(p, x): e  = elu(x)
                h1 = W1 @ e + b1 ; e1 = elu(h1)
                h2 = W2 @ e1 + b2 ; a, g = split(h2)
                gr = x + a * sigmoid(g)
                o  = Wn @ gr + bn          (512, 1024) channel-major
  q = branch(q, x_q); k = branch(k, x_k); v = branch(v, x_k)
  att view: X_att[s, d] = X_cm[s//2, (s%2)*512 + d]  (flat reinterpretation)
  per head n (d = 64n..64n+63):
    scoresT[s2, s1] = sum_d K_att[s2,d] Q_att[s1,d]   (s2 causal blocks)
    eT = exp(scoresT/sqrt(512)) with strict-lower mask (s2 < s1)
    outT[vs, s1] = sum_s2 V_att[s2, 64n+vs] * eT[s2, s1] ; l[s1] = sum_s2 eT
    final[64n+vs, s1] = outT[vs, s1] / l[s1]   (row 0 of l patched to 1)

v2: engine-rebalanced + software-pipelined:
  - branches issued stage-interleaved (h1 v,k,q; e1 v,k,q; ...) so PE
    matmuls of one branch overlap DVE/ACT work of another
  - elu combine / glu mult / gr add / masks moved to gpsimd (was idle)
  - v_aug built by direct SBUF->SBUF DMA (no DRAM roundtrip)
  - softmax normalize: reciprocal (DVE) -> gpsimd partition_broadcast ->
    DVE multiply (no DRAM roundtrips, no big psum->sbuf copy)
"""

import os
import sys
import numpy as np

sys.path.insert(0, "/opt/trn_rl_repo")

C = 256
S = 1024
D = 512
NH = 8
KS = 64
VS = 64
SCALE = 1.0 / float(np.sqrt(512.0))
N_CORES = 8

CFG = {
    "mm_dtype": "bfloat16",  # "float32" | "bfloat16"
    # gpsimd only supports plain tensor_tensor (no scalar-imm ops)
    "elu_combine_engine": "gpsimd",  # dst = me' + r   (me' = min(e,1)-1)
    "glu_mult_engine": "vector",     # u = ha*(1+tg)   (stt needs V)
    "gr_add_engine": "vector",       # gr = u + x
    "mask_engine": "gpsimd",         # eT diag *= mask01
    "fin_engine": "vector",          # fin = pv * rb
    "bcast": "gpsimd",               # rb broadcast: "gpsimd" | "dma"
}


def _register_custom_dve_ops():
    """Register fused DVE ops (runtime extension of dve_ops.OPS):
      ELU_FUSED_ANT: out = select(in1+s0 > 0, in1+s0, min(in0,1)-1)
                     (in0 = exp(in1+s0) from ScalarE; elu in one DVE pass)
      GLU_FUSED_ANT: out = (in1+s0) * (in0+1) * 0.5
                     (in0 = tanh(0.5 g + 0.5 b2g); gated half-sum in one pass)
    """
    from concourse import dve_ops as DO
    from concourse.dve_spec import (
        Spec, Src0, Src1, C0, C1, Zero, One, minn, select, lower,
        _has_src1 as has_src1,
    )
    from concourse.dve_uop import DveOpSpec
    import numpy as np

    if any(op.name == "ELU_FUSED_ANT" for op in DO.OPS):
        return {op.name: op for op in DO.OPS}

    def mk(name, spec):
        opcode = DO._CUSTOM_DVE_ROW_BASE + len(DO.OPS)
        shas = {}
        for ver in ("v3", "v4"):
            s = DveOpSpec(name=name, opcode=opcode,
                          uops=lower(spec, ver=ver), rd1_en=has_src1(spec))
            shas[ver] = s.sha(ver)
        op = DO.DveOp(name, spec, subdim=False, uops_sha=shas)
        DO.OPS.append(op)
        DO.CUSTOM_DVE_SPECS[name] = spec
        DO._SUB_OPCODE_FOR_NAME[name] = opcode
        return op

    t = Src1 + C0
    elu = mk("ELU_FUSED_ANT", Spec(
        body=select(t > Zero, t, minn(Src0, One) - One),
        reference=lambda in0, in1, s0, s1, imm2: np.where(
            in1 + s0 > 0, in1 + s0, np.minimum(in0, 1.0) - 1.0
        ).astype(np.float32),
    ))
    glu = mk("GLU_FUSED_ANT", Spec(
        body=(Src1 + C0) * ((Src0 + One) * C1),
        reference=lambda in0, in1, s0, s1, imm2: (
            (in1 + s0) * (in0 + 1.0) * s1
        ).astype(np.float32),
    ))
    return {"ELU_FUSED_ANT": elu, "GLU_FUSED_ANT": glu}


def _split_psum_ranges(a, b, max_n=512):
    """Split [a, b) psum column range into chunks that don't cross 512-col
    bank boundaries and are <= max_n wide."""
    out = []
    while a < b:
        nxt = min(b, ((a // 512) + 1) * 512, a + max_n)
        out.append((a, nxt))
        a = nxt
    return out


def build_program(cfg=CFG):
    from contextlib import ExitStack

    import concourse.bacc as bacc
    import concourse.bass as bass
    import concourse.tile as tile
    from concourse import mybir
    from concourse.alu_op_type import AluOpType as Op

    f32 = mybir.dt.float32
    mmdt = getattr(mybir.dt, cfg["mm_dtype"])
    mdt = mmdt
    AF = mybir.ActivationFunctionType

    fused = _register_custom_dve_ops()
    ELU_OP = fused["ELU_FUSED_ANT"]
    GLU_OP = fused["GLU_FUSED_ANT"]

    nc = bacc.Bacc("TRN2", target_bir_lowering=False, debug=False,
                   num_devices=N_CORES)

    # ---------------- DRAM parameters ----------------
    idt = mybir.dt.bfloat16 if cfg["mm_dtype"] == "bfloat16" else f32
    query = nc.dram_tensor("query", [C, S], idt, kind="ExternalInput").ap()
    key = nc.dram_tensor("key", [C, S], idt, kind="ExternalInput").ap()
    wT = {}
    bias = {}
    wdt = mdt if mdt == mybir.dt.bfloat16 else f32
    for p in ("q", "k", "v"):
        wT[p, 1] = nc.dram_tensor(f"{p}_w1T", [C, C], wdt, kind="ExternalInput").ap()
        wT[p, 2] = nc.dram_tensor(f"{p}_w2T", [C, 2 * C], wdt, kind="ExternalInput").ap()
        wT[p, "n"] = nc.dram_tensor(f"{p}_wnT", [C, D], wdt, kind="ExternalInput").ap()
        bias[p, 1] = nc.dram_tensor(f"{p}_b1", [C], f32, kind="ExternalInput").ap()
        bias[p, 2] = nc.dram_tensor(f"{p}_b2", [2 * C], f32, kind="ExternalInput").ap()
        bias[p, "n"] = nc.dram_tensor(f"{p}_bn", [D], f32, kind="ExternalInput").ap()
    out_d = nc.dram_tensor("out", [D, S], f32, kind="ExternalOutput").ap()

    def eng(name):
        return getattr(nc, name)

    BR = ("v", "k", "q")  # issue order within stages

    with tile.TileContext(nc) as tc, ExitStack() as ctx:
        # ------------- pools -------------
        persist = ctx.enter_context(tc.tile_pool(name="persist", bufs=1))
        psum_main = ctx.enter_context(tc.tile_pool(name="psum_main", bufs=3, space="PSUM"))
        psum_pv = ctx.enter_context(tc.tile_pool(name="psum_pv", bufs=1, space="PSUM"))
        wk = ctx.enter_context(tc.tile_pool(name="wk", bufs=5))
        big = ctx.enter_context(tc.tile_pool(name="big", bufs=1))
        eT_pool = ctx.enter_context(tc.tile_pool(name="eT", bufs=3))
        att_small = ctx.enter_context(tc.tile_pool(name="att_small", bufs=2))

        # persistent tiles
        xq = persist.tile([128, 2, S], idt)
        xk = persist.tile([128, 2, S], idt)
        eluq = persist.tile([128, 2, S], mdt)
        eluk = persist.tile([128, 2, S], mdt)
        qT_m = persist.tile([128, 4, S], mdt)   # Q^T_att: [dd%128, dd//128, s]
        kT_m = persist.tile([128, 4, S], mdt)
        # [s%128, s//128, n, 1|vs] — ones column FIRST so l lands at psum
        # partition 0 (custom-DVE reciprocal requires base partition 0)
        v_aug = persist.tile([128, 8, NH, VS + 1], mdt)

        # PE warm-up: back-to-back matmuls on scratch data during DMA phase
        warm = persist.tile([128, 512], mdt, name="warm")
        nc.vector.memset(warm, 0.5)
        wps = psum_main.tile([128, 1024], f32, tag="pm", name="wps")
        for _ in range(18):
            nc.tensor.matmul(wps[:, 0:512], lhsT=warm[:, 0:128],
                             rhs=warm, start=True, stop=True)

        # inputs
        for cc in range(2):
            nc.sync.dma_start(out=xk[:, cc, :], in_=key[cc * 128:(cc + 1) * 128, :])
            nc.sync.dma_start(out=xq[:, cc, :], in_=query[cc * 128:(cc + 1) * 128, :])
        bnb = {"q": persist.tile([128, D], f32, name="bnb_q"),
               "k": persist.tile([128, D], f32, name="bnb_k")}
        for p in ("q", "k"):
            bn_ap = bias[p, "n"]
            bn_bcast = bass.AP(tensor=bn_ap.tensor, offset=bn_ap.offset,
                               ap=[[0, 128]] + list(bn_ap.ap))
            nc.sync.dma_start(out=bnb[p], in_=bn_bcast)
        # v_aug ones column (independent of everything; do during DMA phase)
        for j in range(8):
            nc.gpsimd.memset(v_aug[:, j, :, 0:1], 1.0)

        # ------- weights for all branches upfront (persistent) -------
        w1 = {}
        w2 = {}
        wn = {}
        b1 = {}
        b2 = {}
        b2h = {}
        for p in BR:
            w1[p] = persist.tile([128, 2, C], mdt, name=f"w1_{p}")
            w2[p] = persist.tile([128, 2, 2 * C], mdt, name=f"w2_{p}")
            wn[p] = persist.tile([128, 2, D], mdt, name=f"wn_{p}")

            def wcast(ap):
                return ap if ap.dtype == mdt else ap.bitcast(mdt)
            for kc in range(2):
                nc.sync.dma_start(out=w1[p][:, kc, :], in_=wcast(wT[p, 1][kc * 128:(kc + 1) * 128, :]))
                nc.sync.dma_start(out=w2[p][:, kc, :], in_=wcast(wT[p, 2][kc * 128:(kc + 1) * 128, :]))
                nc.sync.dma_start(out=wn[p][:, kc, :], in_=wcast(wT[p, "n"][kc * 128:(kc + 1) * 128, :]))
            b1[p] = persist.tile([128, 2], f32, name=f"b1_{p}")
            b2[p] = persist.tile([128, 4], f32, name=f"b2_{p}")
            b2h[p] = persist.tile([128, 4], f32, name=f"b2h_{p}")
            nc.sync.dma_start(out=b1[p], in_=bias[p, 1].rearrange("(kc p) -> p kc", p=128))
            nc.sync.dma_start(out=b2[p], in_=bias[p, 2].rearrange("(kc p) -> p kc", p=128))
            nc.vector.tensor_scalar_mul(b2h[p], b2[p], 0.5)
        bnv = persist.tile([128, 4], f32, name="bnv")
        nc.sync.dma_start(out=bnv, in_=bias["v", "n"].rearrange("(kc p) -> p kc", p=128))

        # ---------------- branch compute, stage-interleaved ----------------
        # elu(x) for inputs: xk first (feeds v and k), then xq
        def elu_chunk(dst, src_ap, bias_col=None):
            """dst = elu(src [+ bias]) for a (128, S) chunk. src may be psum.
            scalar: e = exp(src + b); vector (fused): dst =
            select(src+b > 0, src+b, min(e,1)-1)."""
            e = wk.tile([128, S], mdt, tag="wke")
            if bias_col is None:
                nc.scalar.activation(e, src_ap, AF.Exp)
                nc.vector._custom_dve(ELU_OP, out=dst, in0=e, in1=src_ap,
                                      s0=0.0)
            else:
                nc.scalar.activation(e, src_ap, AF.Exp, bias=bias_col)
                nc.vector._custom_dve(ELU_OP, out=dst, in0=e, in1=src_ap,
                                      s0=bias_col)

        for cc in range(2):
            elu_chunk(eluk[:, cc, :], xk[:, cc, :])
        for cc in range(2):
            elu_chunk(eluq[:, cc, :], xq[:, cc, :])

        elu3 = {"v": eluk, "k": eluk, "q": eluq}
        x3 = {"v": xk, "k": xk, "q": xq}

        # --- stage 1: h1 = W1 @ elu(x) + b1 ; e1 = elu(h1) ---
        e1 = {p: big.tile([128, 2, S], mdt, name=f"e1_{p}") for p in BR}
        for p in BR:
            for mc in range(2):
                ps = psum_main.tile([128, 1024], f32, tag="pm")
                h1 = ps[:, 0:S]
                for nk in range(2):
                    for kc in range(2):
                        nc.tensor.matmul(
                            h1[:, nk * 512:(nk + 1) * 512],
                            lhsT=w1[p][:, kc, mc * 128:(mc + 1) * 128],
                            rhs=elu3[p][:, kc, nk * 512:(nk + 1) * 512],
                            start=(kc == 0), stop=(kc == 1))
                elu_chunk(e1[p][:, mc, :], h1, bias_col=b1[p][:, mc:mc + 1])

        # --- stage 2: h2 = W2 @ e1 + b2 ; gr = x + 0.5(a+b2a)(1+tanh(0.5(g+b2g))) ---
        gr = {p: big.tile([128, 2, S], mdt, name=f"gr_{p}") for p in BR}
        for p in BR:
            for cc in range(2):
                ps_a = psum_main.tile([128, 1024], f32, tag="pm")
                a_raw = ps_a[:, 0:S]
                for nk in range(2):
                    for kc in range(2):
                        nc.tensor.matmul(
                            a_raw[:, nk * 512:(nk + 1) * 512],
                            lhsT=w2[p][:, kc, cc * 128:(cc + 1) * 128],
                            rhs=e1[p][:, kc, nk * 512:(nk + 1) * 512],
                            start=(kc == 0), stop=(kc == 1))
                ps_g = psum_main.tile([128, 1024], f32, tag="pm")
                g_raw = ps_g[:, 0:S]
                for nk in range(2):
                    for kc in range(2):
                        nc.tensor.matmul(
                            g_raw[:, nk * 512:(nk + 1) * 512],
                            lhsT=w2[p][:, kc, (2 + cc) * 128:(3 + cc) * 128],
                            rhs=e1[p][:, kc, nk * 512:(nk + 1) * 512],
                            start=(kc == 0), stop=(kc == 1))
                tg = wk.tile([128, S], mdt, tag="wke")
                u = wk.tile([128, S], mdt, tag="wku")
                nc.scalar.activation(tg, g_raw, AF.Tanh,
                                     bias=b2h[p][:, 2 + cc:3 + cc], scale=0.5)
                # u = (a + b2a) * 0.5 * (1 + tg)   (fused DVE)
                nc.vector._custom_dve(GLU_OP, out=u, in0=tg, in1=a_raw,
                                      s0=b2[p][:, cc:cc + 1], s1=0.5)
                eng(cfg["gr_add_engine"]).tensor_tensor(
                    gr[p][:, cc, :], u, x3[p][:, cc, :], Op.add)

        # --- stage 3: nin ---
        # v first (feeds v_aug via sbuf->sbuf DMA), then k/q interleaved by
        # hw-block pairs so tp chunks complete in order 0,1,2,3.
        v_sb = big.tile([128, 4, S], mdt, name="v_sb")
        for mc in range(4):
            ps = psum_main.tile([128, 1024], f32, tag="pm")
            vo = ps[:, 0:S]
            for nk in range(2):
                for kc in range(2):
                    nc.tensor.matmul(
                        vo[:, nk * 512:(nk + 1) * 512],
                        lhsT=wn["v"][:, kc, mc * 128:(mc + 1) * 128],
                        rhs=gr["v"][:, kc, nk * 512:(nk + 1) * 512],
                        start=(kc == 0), stop=(kc == 1))
            nc.vector.tensor_scalar(v_sb[:, mc, :], vo, bnv[:, mc:mc + 1], 0.0,
                                    Op.add, Op.add)
            # v_aug[j][p2, n, u] = V_att[128j+p2, 64n+u]; V_att[s, d] =
            # v_cm[s//2, (s%2)*512 + d].  j blocks 2*mc, 2*mc+1 live in chunk mc.
            for j in (2 * mc, 2 * mc + 1):
                src = v_sb[64 * (j % 2):64 * (j % 2) + 64, mc, :]
                src = src.rearrange("c (h n u) -> c h n u", h=2, n=NH)
                nc.sync.dma_start(out=v_aug[:, j, :, 1:VS + 1], in_=src)

        def nin_T(p, hw_p):
            tgt = qT_m if p == "q" else kT_m
            ps = psum_main.tile([128, 1024], f32, tag="pm")
            oT = ps[:, 0:D]
            for kc in range(2):
                nc.tensor.matmul(
                    oT,
                    lhsT=gr[p][:, kc, hw_p * 128:(hw_p + 1) * 128],
                    rhs=wn[p][:, kc, :],
                    start=(kc == 0), stop=(kc == 1))
            tp, jj = hw_p % 4, hw_p // 4
            nc.vector.tensor_tensor(tgt[:, tp, jj::2], oT, bnb[p], Op.add)

        for hw_p in (0, 4, 1, 5, 2, 6, 3, 7):
            nin_T("k", hw_p)
            nin_T("q", hw_p)

        # ---------------- attention ----------------
        # scores psum groups (each <= 1024 cols = 2 banks)
        GROUPS = [(0,), (1, 7), (2, 6), (3, 5), (4,)]
        G = {}
        off = 0
        for grp in GROUPS:
            for j in grp:
                G[j] = off
                off += S - 128 * j

        for n in range(NH):
            tp, po = n // 2, 64 * (n % 2)
            eT = eT_pool.tile([128, 4608], mdt, tag="eT")
            for grp in GROUPS:
                glen = sum(S - 128 * j for j in grp)
                gbase = G[grp[0]]
                ps = psum_main.tile([128, 1024], f32, tag="pm")
                for j in grp:
                    off = G[j] - gbase
                    lhsT = kT_m[po:po + 64, tp, 128 * j:128 * (j + 1)]
                    for s1a, s1b in _split_psum_ranges(off, off + (S - 128 * j)):
                        nc.tensor.matmul(
                            ps[:, s1a:s1b],
                            lhsT=lhsT,
                            rhs=qT_m[po:po + 64, tp,
                                     128 * j + (s1a - off):128 * j + (s1b - off)],
                            start=True, stop=True)
                nc.scalar.activation(eT[:, gbase:gbase + glen],
                                     ps[:, 0:glen], AF.Exp, scale=SCALE)
                for j in grp:
                    # strict-lower mask on the diagonal block, in place:
                    # keep where t1 - t2 - 1 >= 0 else 0
                    nc.gpsimd.affine_select(
                        out=eT[:, G[j]:G[j] + 128], in_=eT[:, G[j]:G[j] + 128],
                        compare_op=Op.is_ge, fill=0.0,
                        base=-1, pattern=[[1, 128]], channel_multiplier=-1)

            pv = psum_pv.tile([VS + 1, 1024], f32, tag="pv")
            for c in range(2):
                jmax = 3 if c == 0 else 7
                for j in range(jmax + 1):
                    s1a = max(512 * c, 128 * j)
                    s1b = 512 * (c + 1)
                    nc.tensor.matmul(
                        pv[:, s1a:s1b],
                        lhsT=v_aug[:, j, n, :],
                        rhs=eT[:, G[j] + (s1a - 128 * j):G[j] + (s1b - 128 * j)],
                        start=(j == 0), stop=(j == jmax))
            # row 0 = l (ones column first); rows 1..64 = unnormalized out
            nc.vector.memset(pv[0:1, 0:1], 1.0)
            # copy whole pv to sbuf immediately (frees the single psum buf),
            # then normalize from sbuf: 1/l -> broadcast -> multiply
            ub = att_small.tile([VS + 1, 1024], f32, tag="ub")
            nc.vector.tensor_copy(ub, pv)
            rrow = att_small.tile([1, 1024], f32, tag="rrow")
            nc.vector.reciprocal_approx_fast(rrow, ub[0:1, :])
            rb = att_small.tile([VS + 1, 1024], f32, tag="rb")
            nc.gpsimd.partition_broadcast(rb, rrow, channels=VS + 1)
            fin = att_small.tile([VS + 1, 1024], f32, tag="fin")
            # row 0 computes l * (1/l); only rows 1..64 are DMA'd out
            eng(cfg["fin_engine"]).tensor_tensor(fin, ub, rb, Op.mult)
            nc.sync.dma_start(out=out_d[VS * n:VS * (n + 1), :],
                              in_=fin[1:VS + 1, :])

    nc.compile()
    return nc


_CACHE = {}


def _get_program(cfg_key=None):
    key = cfg_key or "default"
    if key not in _CACHE:
        _CACHE[key] = build_program(CFG)
    return _CACHE[key]


def make_in_map(inp, b):
    """Per-core input dict for batch b (weights host-transposed/cast)."""
    if CFG["mm_dtype"] == "bfloat16":
        import ml_dtypes
        wt = np.dtype(ml_dtypes.bfloat16)
    else:
        wt = np.float32
    m = {
        "query": np.ascontiguousarray(inp["query"][b].reshape(C, S)).astype(wt),
        "key": np.ascontiguousarray(inp["key"][b].reshape(C, S)).astype(wt),
    }
    for p in ("q", "k", "v"):
        m[f"{p}_w1T"] = np.ascontiguousarray(inp[f"{p}_gr_w1"].T).astype(wt)
        m[f"{p}_w2T"] = np.ascontiguousarray(inp[f"{p}_gr_w2"].T).astype(wt)
        m[f"{p}_wnT"] = np.ascontiguousarray(inp[f"{p}_nin_w"].T).astype(wt)
        m[f"{p}_b1"] = inp[f"{p}_gr_b1"]
        m[f"{p}_b2"] = inp[f"{p}_gr_b2"]
        m[f"{p}_bn"] = inp[f"{p}_nin_b"]
    return m


def kernel(**inputs):
    from concourse.bass_utils import run_bass_kernel_spmd

    nc = _get_program()
    inp = {k: np.asarray(v, dtype=np.float32) for k, v in inputs.items()}

    in_maps = [make_in_map(inp, b) for b in range(N_CORES)]

    trace = bool(int(os.environ.get("BASS_KERNEL_TRACE", "0")))
    res = run_bass_kernel_spmd(nc, in_maps, core_ids=list(range(N_CORES)),
                               trace=trace)
    LAST_RUN["exec_time_ns"] = getattr(res, "exec_time_ns", None)
    LAST_RUN["results"] = res
    out = np.stack([res.results[i]["out"].reshape(D, 32, 32)
                    for i in range(N_CORES)])
    return out.astype(np.float32)


LAST_RUN = {}


if __name__ == "__main__":
    nc = build_program()
    print("compiled OK")


# revision 35
# speedup vs baseline: 1.2268x; 1.0579x over previous
"""Trainium2 Bass kernel for nn_CausalAttention (gated-resnet q/k/v projections
+ causal attention). Data-parallel over batch: 8 batches -> 8 NeuronCores.

Per-core computation (batch b), all fp32 storage:
  x_q = query[b] (C=256, S=1024)   x_k = key[b] (256, 1024)
  branch(p, x): e  = elu(x)
                h1 = W1 @ e + b1 ; e1 = elu(h1)
                h2 = W2 @ e1 + b2 ; a, g = split(h2)
                gr = x + a * sigmoid(g)
                o  = Wn @ gr + bn          (512, 1024) channel-major
  q = branch(q, x_q); k = branch(k, x_k); v = branch(v, x_k)
  att view: X_att[s, d] = X_cm[s//2, (s%2)*512 + d]  (flat reinterpretation)
  per head n (d = 64n..64n+63):
    scoresT[s2, s1] = sum_d K_att[s2,d] Q_att[s1,d]   (s2 causal blocks)
    eT = exp(scoresT/sqrt(512)) with strict-lower mask (s2 < s1)
    outT[vs, s1] = sum_s2 V_att[s2, 64n+vs] * eT[s2, s1] ; l[s1] = sum_s2 eT
    final[64n+vs, s1] = outT[vs, s1] / l[s1]   (row 0 of l patched to 1)

v2: engine-rebalanced + software-pipelined:
  - branches issued stage-interleaved (h1 v,k,q; e1 v,k,q; ...) so PE
    matmuls of one branch overlap DVE/ACT work of another
  - elu combine / glu mult / gr add / masks moved to gpsimd (was idle)
  - v_aug built by direct SBUF->SBUF DMA (no DRAM roundtrip)
  - softmax normalize: reciprocal (DVE) -> gpsimd partition_broadcast ->
    DVE multiply (no DRAM roundtrips, no big psum->sbuf copy)
"""

import os
import sys
import numpy as np

sys.path.insert(0, "/opt/trn_rl_repo")

C = 256
S = 1024
D = 512
NH = 8
KS = 64
VS = 64
SCALE = 1.0 / float(np.sqrt(512.0))
N_CORES = 8

CFG = {
    "mm_dtype": "bfloat16",  # "float32" | "bfloat16"
    # gpsimd only supports plain tensor_tensor (no scalar-imm ops)
    "elu_combine_engine": "gpsimd",  # dst = me' + r   (me' = min(e,1)-1)
    "glu_mult_engine": "vector",     # u = ha*(1+tg)   (stt needs V)
    "gr_add_engine": "gpsimd",       # gr = u + x (G idle in branch phase)
    "mask_engine": "gpsimd",         # eT diag *= mask01
    "fin_engine": "vector",          # fin = pv * rb
    "bcast": "dma",                  # rb broadcast: "gpsimd" | "dma"
}


def _register_custom_dve_ops():
    """Register fused DVE ops (runtime extension of dve_ops.OPS):
      ELU_FUSED_ANT: out = select(in1+s0 > 0, in1+s0, min(in0,1)-1)
                     (in0 = exp(in1+s0) from ScalarE; elu in one DVE pass)
      GLU_FUSED_ANT: out = (in1+s0) * (in0+1) * 0.5
                     (in0 = tanh(0.5 g + 0.5 b2g); gated half-sum in one pass)
    """
    from concourse import dve_ops as DO
    from concourse.dve_spec import (
        Spec, Src0, Src1, C0, C1, Zero, One, minn, select, lower,
        _has_src1 as has_src1,
    )
    from concourse.dve_uop import DveOpSpec
    import numpy as np

    if any(op.name == "ELU_FUSED_ANT" for op in DO.OPS):
        return {op.name: op for op in DO.OPS}

    def mk(name, spec):
        opcode = DO._CUSTOM_DVE_ROW_BASE + len(DO.OPS)
        shas = {}
        for ver in ("v3", "v4"):
            s = DveOpSpec(name=name, opcode=opcode,
                          uops=lower(spec, ver=ver), rd1_en=has_src1(spec))
            shas[ver] = s.sha(ver)
        op = DO.DveOp(name, spec, subdim=False, uops_sha=shas)
        DO.OPS.append(op)
        DO.CUSTOM_DVE_SPECS[name] = spec
        DO._SUB_OPCODE_FOR_NAME[name] = opcode
        return op

    t = Src1 + C0
    elu = mk("ELU_FUSED_ANT", Spec(
        body=select(t > Zero, t, minn(Src0, One) - One),
        reference=lambda in0, in1, s0, s1, imm2: np.where(
            in1 + s0 > 0, in1 + s0, np.minimum(in0, 1.0) - 1.0
        ).astype(np.float32),
    ))
    glu = mk("GLU_FUSED_ANT", Spec(
        body=(Src1 + C0) * ((Src0 + One) * C1),
        reference=lambda in0, in1, s0, s1, imm2: (
            (in1 + s0) * (in0 + 1.0) * s1
        ).astype(np.float32),
    ))
    return {"ELU_FUSED_ANT": elu, "GLU_FUSED_ANT": glu}


def _split_psum_ranges(a, b, max_n=512):
    """Split [a, b) psum column range into chunks that don't cross 512-col
    bank boundaries and are <= max_n wide."""
    out = []
    while a < b:
        nxt = min(b, ((a // 512) + 1) * 512, a + max_n)
        out.append((a, nxt))
        a = nxt
    return out


def build_program(cfg=CFG):
    from contextlib import ExitStack

    import concourse.bacc as bacc
    import concourse.bass as bass
    import concourse.tile as tile
    from concourse import mybir
    from concourse.alu_op_type import AluOpType as Op

    f32 = mybir.dt.float32
    mmdt = getattr(mybir.dt, cfg["mm_dtype"])
    mdt = mmdt
    AF = mybir.ActivationFunctionType

    fused = _register_custom_dve_ops()
    ELU_OP = fused["ELU_FUSED_ANT"]
    GLU_OP = fused["GLU_FUSED_ANT"]

    nc = bacc.Bacc("TRN2", target_bir_lowering=False, debug=False,
                   num_devices=N_CORES)

    # ---------------- DRAM parameters ----------------
    idt = mybir.dt.bfloat16 if cfg["mm_dtype"] == "bfloat16" else f32
    query = nc.dram_tensor("query", [C, S], idt, kind="ExternalInput").ap()
    key = nc.dram_tensor("key", [C, S], idt, kind="ExternalInput").ap()
    wT = {}
    bias = {}
    wdt = mdt if mdt == mybir.dt.bfloat16 else f32
    for p in ("q", "k", "v"):
        wT[p, 1] = nc.dram_tensor(f"{p}_w1T", [C, C], wdt, kind="ExternalInput").ap()
        wT[p, 2] = nc.dram_tensor(f"{p}_w2T", [C, 2 * C], wdt, kind="ExternalInput").ap()
        wT[p, "n"] = nc.dram_tensor(f"{p}_wnT", [C, D], wdt, kind="ExternalInput").ap()
        bias[p, 1] = nc.dram_tensor(f"{p}_b1", [C], f32, kind="ExternalInput").ap()
        bias[p, 2] = nc.dram_tensor(f"{p}_b2", [2 * C], f32, kind="ExternalInput").ap()
        bias[p, "n"] = nc.dram_tensor(f"{p}_bn", [D], f32, kind="ExternalInput").ap()
    out_d = nc.dram_tensor("out", [D, S], f32, kind="ExternalOutput").ap()

    def eng(name):
        return getattr(nc, name)

    BR = ("v", "k", "q")  # issue order within stages

    with tile.TileContext(nc) as tc, ExitStack() as ctx:
        # ------------- pools -------------
        persist = ctx.enter_context(tc.tile_pool(name="persist", bufs=1))
        psum_main = ctx.enter_context(tc.tile_pool(name="psum_main", bufs=3, space="PSUM"))
        psum_pv = ctx.enter_context(tc.tile_pool(name="psum_pv", bufs=1, space="PSUM"))
        dram_pool = ctx.enter_context(tc.tile_pool(name="dram", bufs=1, space="DRAM"))
        wk = ctx.enter_context(tc.tile_pool(name="wk", bufs=5))
        big = ctx.enter_context(tc.tile_pool(name="big", bufs=1))
        eT_pool = ctx.enter_context(tc.tile_pool(name="eT", bufs=3))
        att_small = ctx.enter_context(tc.tile_pool(name="att_small", bufs=2))

        # persistent tiles
        xq = persist.tile([128, 2, S], idt)
        xk = persist.tile([128, 2, S], idt)
        eluq = persist.tile([128, 2, S], mdt)
        eluk = persist.tile([128, 2, S], mdt)
        qT_m = persist.tile([128, 4, S], mdt)   # Q^T_att: [dd%128, dd//128, s]
        kT_m = persist.tile([128, 4, S], mdt)
        # [s%128, s//128, n, 1|vs] — ones column FIRST so l lands at psum
        # partition 0 (custom-DVE reciprocal requires base partition 0)
        v_aug = persist.tile([128, 8, NH, VS + 1], mdt)

        # PE warm-up: back-to-back matmuls on scratch data during DMA phase
        warm = persist.tile([128, 512], mdt, name="warm")
        nc.vector.memset(warm, 0.5)
        # preload the exp ACT table set during the DMA phase (one-time ~2.7us)
        warm_act = persist.tile([128, 1], f32, name="warm_act")
        nc.scalar.activation(warm_act, warm[:, 0:1], AF.Exp)
        wps = psum_main.tile([128, 1024], f32, tag="pm", name="wps")
        for _ in range(18):
            nc.tensor.matmul(wps[:, 0:512], lhsT=warm[:, 0:128],
                             rhs=warm, start=True, stop=True)

        # inputs
        for cc in range(2):
            nc.sync.dma_start(out=xk[:, cc, :], in_=key[cc * 128:(cc + 1) * 128, :])
            nc.sync.dma_start(out=xq[:, cc, :], in_=query[cc * 128:(cc + 1) * 128, :])
        bnb = {"q": persist.tile([128, D], f32, name="bnb_q"),
               "k": persist.tile([128, D], f32, name="bnb_k")}
        for p in ("q", "k"):
            bn_ap = bias[p, "n"]
            bn_bcast = bass.AP(tensor=bn_ap.tensor, offset=bn_ap.offset,
                               ap=[[0, 128]] + list(bn_ap.ap))
            nc.sync.dma_start(out=bnb[p], in_=bn_bcast)
        # v_aug ones column (independent of everything; do during DMA phase)
        for j in range(8):
            nc.gpsimd.memset(v_aug[:, j, :, 0:1], 1.0)

        # ------- weights for all branches upfront (persistent) -------
        w1 = {}
        w2 = {}
        wn = {}
        b1 = {}
        b2 = {}
        b2h = {}
        for p in BR:
            w1[p] = persist.tile([128, 2, C], mdt, name=f"w1_{p}")
            w2[p] = persist.tile([128, 2, 2 * C], mdt, name=f"w2_{p}")
            wn[p] = persist.tile([128, 2, D], mdt, name=f"wn_{p}")

            def wcast(ap):
                return ap if ap.dtype == mdt else ap.bitcast(mdt)
            for kc in range(2):
                nc.sync.dma_start(out=w1[p][:, kc, :], in_=wcast(wT[p, 1][kc * 128:(kc + 1) * 128, :]))
                nc.sync.dma_start(out=w2[p][:, kc, :], in_=wcast(wT[p, 2][kc * 128:(kc + 1) * 128, :]))
                nc.sync.dma_start(out=wn[p][:, kc, :], in_=wcast(wT[p, "n"][kc * 128:(kc + 1) * 128, :]))
            b1[p] = persist.tile([128, 2], f32, name=f"b1_{p}")
            b2[p] = persist.tile([128, 4], f32, name=f"b2_{p}")
            b2h[p] = persist.tile([128, 4], f32, name=f"b2h_{p}")
            nc.sync.dma_start(out=b1[p], in_=bias[p, 1].rearrange("(kc p) -> p kc", p=128))
            nc.sync.dma_start(out=b2[p], in_=bias[p, 2].rearrange("(kc p) -> p kc", p=128))
            nc.vector.tensor_scalar_mul(b2h[p], b2[p], 0.5)
        bnv = persist.tile([128, 4], f32, name="bnv")
        nc.sync.dma_start(out=bnv, in_=bias["v", "n"].rearrange("(kc p) -> p kc", p=128))

        # ---------------- branch compute, stage-interleaved ----------------
        # elu(x) for inputs: xk first (feeds v and k), then xq
        def elu_chunk(dst, src_ap, bias_col=None):
            """dst = elu(src [+ bias]) for a (128, S) chunk. src may be psum.
            scalar: e = exp(src + b); vector (fused): dst =
            select(src+b > 0, src+b, min(e,1)-1)."""
            e = wk.tile([128, S], mdt, tag="wke")
            if bias_col is None:
                nc.scalar.activation(e, src_ap, AF.Exp)
                nc.vector._custom_dve(ELU_OP, out=dst, in0=e, in1=src_ap,
                                      s0=0.0)
            else:
                nc.scalar.activation(e, src_ap, AF.Exp, bias=bias_col)
                nc.vector._custom_dve(ELU_OP, out=dst, in0=e, in1=src_ap,
                                      s0=bias_col)

        for cc in range(2):
            elu_chunk(eluk[:, cc, :], xk[:, cc, :])
        for cc in range(2):
            elu_chunk(eluq[:, cc, :], xq[:, cc, :])

        elu3 = {"v": eluk, "k": eluk, "q": eluq}
        x3 = {"v": xk, "k": xk, "q": xq}

        # --- stage 1: h1 = W1 @ elu(x) + b1 ; e1 = elu(h1) ---
        e1 = {p: big.tile([128, 2, S], mdt, name=f"e1_{p}") for p in BR}
        for p in BR:
            for mc in range(2):
                ps = psum_main.tile([128, 1024], f32, tag="pm")
                h1 = ps[:, 0:S]
                for nk in range(2):
                    for kc in range(2):
                        nc.tensor.matmul(
                            h1[:, nk * 512:(nk + 1) * 512],
                            lhsT=w1[p][:, kc, mc * 128:(mc + 1) * 128],
                            rhs=elu3[p][:, kc, nk * 512:(nk + 1) * 512],
                            start=(kc == 0), stop=(kc == 1))
                elu_chunk(e1[p][:, mc, :], h1, bias_col=b1[p][:, mc:mc + 1])

        # --- stage 2: h2 = W2 @ e1 + b2 ; gr = x + 0.5(a+b2a)(1+tanh(0.5(g+b2g))) ---
        gr = {p: big.tile([128, 2, S], mdt, name=f"gr_{p}") for p in BR}
        for p in BR:
            for cc in range(2):
                ps_a = psum_main.tile([128, 1024], f32, tag="pm")
                a_raw = ps_a[:, 0:S]
                for nk in range(2):
                    for kc in range(2):
                        nc.tensor.matmul(
                            a_raw[:, nk * 512:(nk + 1) * 512],
                            lhsT=w2[p][:, kc, cc * 128:(cc + 1) * 128],
                            rhs=e1[p][:, kc, nk * 512:(nk + 1) * 512],
                            start=(kc == 0), stop=(kc == 1))
                ps_g = psum_main.tile([128, 1024], f32, tag="pm")
                g_raw = ps_g[:, 0:S]
                for nk in range(2):
                    for kc in range(2):
                        nc.tensor.matmul(
                            g_raw[:, nk * 512:(nk + 1) * 512],
                            lhsT=w2[p][:, kc, (2 + cc) * 128:(3 + cc) * 128],
                            rhs=e1[p][:, kc, nk * 512:(nk + 1) * 512],
                            start=(kc == 0), stop=(kc == 1))
                tg = wk.tile([128, S], mdt, tag="wke")
                u = wk.tile([128, S], mdt, tag="wku")
                nc.scalar.activation(tg, g_raw, AF.Tanh,
                                     bias=b2h[p][:, 2 + cc:3 + cc], scale=0.5)
                # u = (a + b2a) * 0.5 * (1 + tg)   (fused DVE)
                nc.vector._custom_dve(GLU_OP, out=u, in0=tg, in1=a_raw,
                                      s0=b2[p][:, cc:cc + 1], s1=0.5)
                eng(cfg["gr_add_engine"]).tensor_tensor(
                    gr[p][:, cc, :], u, x3[p][:, cc, :], Op.add)

        # --- stage 3: nin ---
        # v first (feeds v_aug via sbuf->sbuf DMA), then k/q interleaved by
        # hw-block pairs so tp chunks complete in order 0,1,2,3.
        v_sb = big.tile([128, 4, S], mdt, name="v_sb")
        for mc in range(4):
            ps = psum_main.tile([128, 1024], f32, tag="pm")
            vo = ps[:, 0:S]
            for nk in range(2):
                for kc in range(2):
                    nc.tensor.matmul(
                        vo[:, nk * 512:(nk + 1) * 512],
                        lhsT=wn["v"][:, kc, mc * 128:(mc + 1) * 128],
                        rhs=gr["v"][:, kc, nk * 512:(nk + 1) * 512],
                        start=(kc == 0), stop=(kc == 1))
            nc.scalar.activation(v_sb[:, mc, :], vo, AF.Identity,
                                 bias=bnv[:, mc:mc + 1])
            # v_aug[j][p2, n, u] = V_att[128j+p2, 64n+u]; V_att[s, d] =
            # v_cm[s//2, (s%2)*512 + d].  j blocks 2*mc, 2*mc+1 live in chunk mc.
            for j in (2 * mc, 2 * mc + 1):
                src = v_sb[64 * (j % 2):64 * (j % 2) + 64, mc, :]
                src = src.rearrange("c (h n u) -> c h n u", h=2, n=NH)
                nc.sync.dma_start(out=v_aug[:, j, :, 1:VS + 1], in_=src)

        def nin_T(p, hw_p):
            tgt = qT_m if p == "q" else kT_m
            ps = psum_main.tile([128, 1024], f32, tag="pm")
            oT = ps[:, 0:D]
            for kc in range(2):
                nc.tensor.matmul(
                    oT,
                    lhsT=gr[p][:, kc, hw_p * 128:(hw_p + 1) * 128],
                    rhs=wn[p][:, kc, :],
                    start=(kc == 0), stop=(kc == 1))
            tp, jj = hw_p % 4, hw_p // 4
            nc.vector.tensor_tensor(tgt[:, tp, jj::2], oT, bnb[p], Op.add)

        for hw_p in (0, 4, 1, 5, 2, 6, 3, 7):
            nin_T("k", hw_p)
            nin_T("q", hw_p)

        # ---------------- attention ----------------
        # scores psum groups (each <= 1024 cols = 2 banks)
        GROUPS = [(0,), (1, 7), (2, 6), (3, 5), (4,)]
        G = {}
        off = 0
        for grp in GROUPS:
            for j in grp:
                G[j] = off
                off += S - 128 * j

        recip_dram = dram_pool.tile([NH, 1024], f32)
        for n in range(NH):
            tp, po = n // 2, 64 * (n % 2)
            eT = eT_pool.tile([128, 4608], mdt, tag="eT")
            for grp in GROUPS:
                glen = sum(S - 128 * j for j in grp)
                gbase = G[grp[0]]
                ps = psum_main.tile([128, 1024], f32, tag="pm")
                for j in grp:
                    off = G[j] - gbase
                    lhsT = kT_m[po:po + 64, tp, 128 * j:128 * (j + 1)]
                    for s1a, s1b in _split_psum_ranges(off, off + (S - 128 * j)):
                        nc.tensor.matmul(
                            ps[:, s1a:s1b],
                            lhsT=lhsT,
                            rhs=qT_m[po:po + 64, tp,
                                     128 * j + (s1a - off):128 * j + (s1b - off)],
                            start=True, stop=True)
                nc.scalar.activation(eT[:, gbase:gbase + glen],
                                     ps[:, 0:glen], AF.Exp, scale=SCALE)
                for j in grp:
                    # strict-lower mask on the diagonal block, in place:
                    # keep where t1 - t2 - 1 >= 0 else 0
                    nc.gpsimd.affine_select(
                        out=eT[:, G[j]:G[j] + 128], in_=eT[:, G[j]:G[j] + 128],
                        compare_op=Op.is_ge, fill=0.0,
                        base=-1, pattern=[[1, 128]], channel_multiplier=-1)

            pv = psum_pv.tile([VS + 1, 1024], f32, tag="pv")
            for c in range(2):
                jmax = 3 if c == 0 else 7
                for j in range(jmax + 1):
                    s1a = max(512 * c, 128 * j)
                    s1b = 512 * (c + 1)
                    nc.tensor.matmul(
                        pv[:, s1a:s1b],
                        lhsT=v_aug[:, j, n, :],
                        rhs=eT[:, G[j] + (s1a - 128 * j):G[j] + (s1b - 128 * j)],
                        start=(j == 0), stop=(j == jmax))
            # row 0 = l (ones column first); rows 1..64 = unnormalized out
            nc.vector.memset(pv[0:1, 0:1], 1.0)
            # copy whole pv to sbuf immediately (frees the single psum buf),
            # then normalize from sbuf: 1/l -> broadcast -> multiply
            ub = att_small.tile([VS + 1, 1024], f32, tag="ub")
            nc.vector.tensor_copy(ub, pv)
            rrow = att_small.tile([1, 1024], f32, tag="rrow")
            nc.vector.reciprocal_approx_fast(rrow, ub[0:1, :])
            rb = att_small.tile([VS + 1, 1024], f32, tag="rb")
            if cfg["bcast"] == "gpsimd":
                nc.gpsimd.partition_broadcast(rb, rrow, channels=VS + 1)
            else:
                # broadcast via DRAM scratch (flat APs allow 0-stride reads)
                nc.sync.dma_start(out=recip_dram[n:n + 1, :], in_=rrow)
                rd = recip_dram.rearrange("a b -> (a b)")[1024 * n:1024 * (n + 1)]
                rsrc = bass.AP(tensor=rd.tensor, offset=rd.offset,
                               ap=[[0, VS + 1]] + list(rd.ap))
                nc.sync.dma_start(out=rb, in_=rsrc)
            fin = att_small.tile([VS + 1, 1024], f32, tag="fin")
            # row 0 computes l * (1/l); only rows 1..64 are DMA'd out
            eng(cfg["fin_engine"]).tensor_tensor(fin, ub, rb, Op.mult)
            nc.sync.dma_start(out=out_d[VS * n:VS * (n + 1), :],
                              in_=fin[1:VS + 1, :])

    nc.compile()
    return nc


_CACHE = {}


def _get_program(cfg_key=None):
    key = cfg_key or "default"
    if key not in _CACHE:
        _CACHE[key] = build_program(CFG)
    return _CACHE[key]


def make_in_map(inp, b):
    """Per-core input dict for batch b (weights host-transposed/cast)."""
    if CFG["mm_dtype"] == "bfloat16":
        import ml_dtypes
        wt = np.dtype(ml_dtypes.bfloat16)
    else:
        wt = np.float32
    m = {
        "query": np.ascontiguousarray(inp["query"][b].reshape(C, S)).astype(wt),
        "key": np.ascontiguousarray(inp["key"][b].reshape(C, S)).astype(wt),
    }
    for p in ("q", "k", "v"):
        m[f"{p}_w1T"] = np.ascontiguousarray(inp[f"{p}_gr_w1"].T).astype(wt)
        m[f"{p}_w2T"] = np.ascontiguousarray(inp[f"{p}_gr_w2"].T).astype(wt)
        m[f"{p}_wnT"] = np.ascontiguousarray(inp[f"{p}_nin_w"].T).astype(wt)
        m[f"{p}_b1"] = inp[f"{p}_gr_b1"]
        m[f"{p}_b2"] = inp[f"{p}_gr_b2"]
        m[f"{p}_bn"] = inp[f"{p}_nin_b"]
    return m


def kernel(**inputs):
    from concourse.bass_utils import run_bass_kernel_spmd

    nc = _get_program()
    inp = {k: np.asarray(v, dtype=np.float32) for k, v in inputs.items()}

    in_maps = [make_in_map(inp, b) for b in range(N_CORES)]

    trace = bool(int(os.environ.get("BASS_KERNEL_TRACE", "0")))
    res = run_bass_kernel_spmd(nc, in_maps, core_ids=list(range(N_CORES)),
                               trace=trace)
    LAST_RUN["exec_time_ns"] = getattr(res, "exec_time_ns", None)
    LAST_RUN["results"] = res
    out = np.stack([res.results[i]["out"].reshape(D, 32, 32)
                    for i in range(N_CORES)])
    return out.astype(np.float32)


LAST_RUN = {}


if __name__ == "__main__":
    nc = build_program()
    print("compiled OK")
